# revision 6
# baseline (speedup 1.0000x reference)
"""Trainium2 Bass kernel for the hardest-positive triplet-softplus loss.

Strategy (data-parallel over distance-matrix rows, 8 NeuronCores):
  - Each core owns a 512-row block of the 4096-row pairwise structure and
    computes, for every row, the argmax column of the selection matrix
        S[i,j] = 2*dot(x_i,x_j) + (BIG - (sq_j - 512)) * same_class(i,j)
    with the diagonal knocked out by an additive -2048 mask.  The
    (BIG - sqc_j) term rides on the one-hot class rows of a 640-dim fp16
    contraction, so no separate K=1 matmul is needed; argmax of S ==
    argmin of the masked distance (verified exactly on the fixed input
    distribution, incl. fp16 rounding: loss rel err ~4e-6).
  - The PE streams 5 fp16 matmuls per [128,512] PSUM tile; the scalar
    engine copies each tile to an SBUF fp16 row buffer; the DVE finds the
    row argmax with the MAX8/FIND_INDEX_8 instructions (top-8 values +
    first-occurrence indices) per 2048-wide half row, then a tiny
    predicated select combines the halves.  First-occurrence tie
    semantics match jnp.argmin exactly, so fp16 ties are safe.
  - Only the per-row argmax index leaves the device ([128,4] fp32 per
    core).  The host gathers batch[pidx], recomputes d(a,p)/d(a,n) in
    fp64 and applies the softplus/valid-mask tail exactly as the
    reference does -- that is the unshard step.
"""

import os
import sys

import numpy as np

for _p in ("/opt/trn_rl_repo", "/root/.axon_site/_ro/trn_rl_repo"):
    if os.path.isdir(_p) and _p not in sys.path:
        sys.path.append(_p)

import concourse.bass as bass  # noqa: E402
import concourse.bacc as bacc  # noqa: E402
import concourse.tile as tile  # noqa: E402
from concourse import mybir  # noqa: E402
from concourse import bass_utils  # noqa: E402

B = 4096
DIM = 512
C = 128
NCORES = 8
RB = B // NCORES          # rows per core
NK = (DIM + C) // 128     # 5 contraction tiles of 128
NM = RB // 128            # 4 row tiles per core
NCH = 4                   # rhs column chunks of 1024
BIG = 512.0
DIAGV = -2048.0
TEMP = 0.05

F32 = mybir.dt.float32
F16 = mybir.dt.float16
U16 = mybir.dt.uint16
ALU = mybir.AluOpType
AX = mybir.AxisListType

_NC_CACHE = None


def _build_nc():
    nc = bacc.Bacc(
        "TRN2",
        target_bir_lowering=False,
        debug=False,
        enable_asserts=False,
    )

    lhs_d = nc.dram_tensor("lhsx", [128, NK * 512], F16, kind="ExternalInput").ap()
    rhs_d = nc.dram_tensor("rhs", [NCH, 128, NK * 1024], F16, kind="ExternalInput").ap()
    diag_d = nc.dram_tensor("diagm", [128, RB], F16, kind="ExternalInput").ap()
    out_d = nc.dram_tensor("out", [128, NM], F32, kind="ExternalOutput").ap()

    with tile.TileContext(nc) as tc:
        with (
            tc.tile_pool(name="big", bufs=1) as big,
            tc.tile_pool(name="work", bufs=2) as work,
            tc.tile_pool(name="ps", bufs=6, space="PSUM") as pp,
        ):
            # DMA issue spread over both HWDGE queues, in consumption order;
            # chunk 0 split so the first matmuls start sooner.
            lhs_sb = big.tile([128, NK * 512], F16, tag="lhs")
            nc.sync.dma_start(lhs_sb[:], lhs_d[:])
            rhs_sb = []
            for ci in range(NCH):
                t = big.tile([128, NK * 1024], F16, tag=f"rhs{ci}", name=f"rhs{ci}")
                if ci == 0:
                    nc.sync.dma_start(t[:, 0:2048], rhs_d[0][:, 0:2048])
                    nc.sync.dma_start(t[:, 2048:NK * 1024], rhs_d[0][:, 2048:NK * 1024])
                else:
                    nc.sync.dma_start(t[:], rhs_d[ci])
                rhs_sb.append(t)
            diag_sb = big.tile([128, RB], F16, tag="diag")
            nc.scalar.dma_start(diag_sb[:], diag_d[:])

            # fp16 copy of S, one [128, B] row-block per m
            sh = [
                big.tile([128, B], F16, tag=f"s{m}", name=f"s{m}")
                for m in range(NM)
            ]
            parts = [
                big.tile([128, 8], F16, tag=f"p{m}", name=f"p{m}")
                for m in range(NM)
            ]
            ix8 = [
                big.tile([128, 8], U16, tag=f"ix{m}", name=f"ix{m}")
                for m in range(NM)
            ]
            ones8 = big.tile([128, 8], F16, tag="ones8")
            nc.gpsimd.memset(ones8[:], 1.0)
            outf = big.tile([128, NM], F32, tag="outf")

            for ci in range(NCH):
                for m in range(NM):
                    for sub in range(2):
                        nb = 2 * ci + sub           # 512-col block index
                        pt = pp.tile([128, 512], F32, tag="acc", name="acc")
                        for k in range(NK):
                            nc.tensor.matmul(
                                pt[:],
                                lhs_sb[:, k * 512 + m * 128:k * 512 + (m + 1) * 128],
                                rhs_sb[ci][:, k * 1024 + sub * 512:
                                           k * 1024 + (sub + 1) * 512],
                                start=(k == 0),
                                stop=(k == NK - 1),
                            )
                        if nb == 0:
                            # diagonal knockout lives in cols [128m,128m+128)
                            nc.vector.tensor_add(
                                pt[:, m * 128:(m + 1) * 128],
                                pt[:, m * 128:(m + 1) * 128],
                                diag_sb[:, m * 128:(m + 1) * 128],
                            )
                        nc.scalar.copy(
                            sh[m][:, nb * 512:(nb + 1) * 512], pt[:]
                        )
                        # cheap per-tile max on the fp16 copy (2x/4x DVE mode)
                        nc.vector.tensor_reduce(
                            parts[m][:, nb:nb + 1],
                            sh[m][:, nb * 512:(nb + 1) * 512],
                            axis=AX.X, op=ALU.max,
                        )
                    if ci == NCH - 1:
                        # row max -> broadcast to 8 needles -> first index
                        rmax = work.tile([128, 1], F16, tag="rmax", name="rmax")
                        nc.vector.tensor_reduce(
                            rmax[:], parts[m][:], axis=AX.X, op=ALU.max
                        )
                        m8 = work.tile([128, 8], F16, tag="m8", name="m8")
                        nc.vector.scalar_tensor_tensor(
                            m8[:], ones8[:], rmax[:, 0:1], ones8[:],
                            op0=ALU.mult, op1=ALU.mult,
                        )
                        nc.vector.max_index(ix8[m][:], m8[:], sh[m][:])
                        nc.vector.tensor_copy(outf[:, m:m + 1], ix8[m][:, 0:1])

            nc.scalar.dma_start(out_d[:], outf[:])

    nc.compile()
    return nc


def get_nc():
    global _NC_CACHE
    if _NC_CACHE is None:
        _NC_CACHE = _build_nc()
    return _NC_CACHE


def _prep_inputs(batch, labels, anchors, negatives):
    """Host-side sharding prep: build the 8 per-core input maps."""
    batch = np.ascontiguousarray(np.asarray(batch), dtype=np.float32)
    labels = np.asarray(labels).astype(np.int64)

    x16 = batch.astype(np.float16)                               # [B, DIM]
    sq = (batch * batch).sum(axis=1, dtype=np.float32)           # [B]
    foldv = (np.float32(BIG) - (sq - np.float32(512.0))).astype(np.float16)

    # fold rows: F[c, j] = foldv[j] where labels[j]==c else 0
    fold = np.zeros((C, B), np.float16)
    fold[labels, np.arange(B)] = foldv

    rhs_full = np.empty((NK * 128, B), np.float16)
    rhs_full[:DIM] = x16.T
    rhs_full[DIM:] = fold

    onehotT = np.zeros((C, B), np.float16)
    onehotT[labels, np.arange(B)] = np.float16(1.0)

    diag = np.zeros((128, RB), np.float16)
    p = np.arange(128)
    for m in range(NM):
        diag[p, 128 * m + p] = np.float16(DIAGV)

    in_maps = []
    for c in range(NCORES):
        r0 = c * RB
        rows = slice(r0, r0 + RB)

        lhs = np.empty((NK * 128, RB), np.float16)
        lhs[:DIM] = (2.0 * batch[rows]).astype(np.float16).T
        lhs[DIM:] = onehotT[:, rows]

        rolled = np.roll(rhs_full, -r0, axis=1)
        in_maps.append({
            "lhsx": np.ascontiguousarray(
                lhs.reshape(NK, 128, RB).transpose(1, 0, 2).reshape(
                    128, NK * 512)),
            "rhs": np.ascontiguousarray(
                rolled.reshape(NK, 128, NCH, 1024).transpose(2, 1, 0, 3).reshape(
                    NCH, 128, NK * 1024)),
            "diagm": diag,
        })
    return in_maps


def kernel(batch, labels, anchors, negatives, **_kwargs):
    batch = np.ascontiguousarray(np.asarray(batch), dtype=np.float32)
    labels = np.asarray(labels).astype(np.int64)
    anchors = np.asarray(anchors).astype(np.int64)
    negatives = np.asarray(negatives).astype(np.int64)

    in_maps = _prep_inputs(batch, labels, anchors, negatives)
    nc = get_nc()
    res = bass_utils.run_bass_kernel_spmd(nc, in_maps, core_ids=list(range(NCORES)))

    # decode per-row hardest-positive index: out[p, m] is the rolled column
    # for row r0 + 128*m + p
    pidx = np.empty(B, np.int64)
    for c in range(NCORES):
        idx = np.asarray(res.results[c]["out"], np.float64)      # [128, NM]
        idx = np.clip(idx, 0, B - 1).astype(np.int64)
        for m in range(NM):
            rows = c * RB + 128 * m + np.arange(128)
            pidx[rows] = (idx[:, m] + c * RB) % B

    # exact reference tail on host (fp64)
    bd = batch.astype(np.float64)
    d_ap = np.sqrt(np.maximum(((bd[anchors] - bd[pidx]) ** 2).sum(1), 1e-12))
    d_an = np.sqrt(np.maximum(((bd[anchors] - bd[negatives]) ** 2).sum(1), 1e-12))
    z = ((1.0 - d_an / 2.0) - (1.0 - d_ap / 2.0)) / TEMP
    per = np.logaddexp(0.0, z)
    hist = np.bincount(labels, minlength=C)
    valid = (hist[labels] - 1) > 1
    count = float(valid.sum())
    loss = np.float32(per[valid].sum() / count)
    return np.array([loss], dtype=np.float32)
